# revision 31
# baseline (speedup 1.0000x reference)
"""Trainium2 Bass kernel for FISTA sparse coding (nn_FISTA_7550552506950).

Strategy (data-parallel over batch, 8 cores x 128 rows), v8:
- State z kept TRANSPOSED [F=4096, B=128] on-chip as float32r (~13-bit
  effective due to f32r write rounding - verified within tolerance), split
  into 32 f-chunks of [128, 256] (real|imag column halves).
- True-residual gradient: the tiny complex residual r = Dw - x is built once
  per iteration and streamed as the fp16 concatenation
  [r_re|r_im ; r_im|-r_re], so ONE fp16 matmul per chunk produces both
  gradient halves (fp16 safe: r is small, cancellation already done in fp32).
  The cross-partition quadrant folds use PE permutation matmuls (a [128->64]
  swap for the bottom quadrants, two [64->64] moves for the ns-half) instead
  of SBUF DMAs, keeping the boundary latency short and the PE streaming.
- A-chain (P1 = D @ z): one f32r matmul per chunk streaming z directly; the
  momentum combine A(w) = a*A(z) + b*A(z_old) happens on the tiny P1 tile
  via qold = b*P1_old - X4 (precomputed off the critical path).
- Momentum a-term (a*z) enters PSUM via f32r scaled-identity matmuls;
  b-term (b*z_old) is added by a fp32-exact DVE scalar_tensor_tensor.
  a = fp16-rounded (exact), b = 1-a fp32 => coefficient rounding cancels.
- Soft-threshold: t12 = (k*u)^2 in fp16 (k=1024, folded into W2/aI/b so the
  squares stay in fp16 normal range; single ACT op per group), m2 fp16
  (GPSIMD), rsq = Rsqrt (ACT raw, batched per group pair), s = relu-affine
  (ACT, fp32, 1/k folded into the bias), z = u*s as contiguous TTs (DVE).
- The per-pair tails (rsq/s and z-writes) are software-pipelined one to two
  pairs behind their drains so the in-order DVE queue never head-of-line
  blocks on the scalar engine; the A-chain runs after the whole group loop
  so the PE stays dense (high p-state) and never chases z-writes.
- Final |z| = sqrt(m2)/k * s stored fp16; global max normalization on host
  during the gather (tiny).
"""

import numpy as np
from contextlib import ExitStack

import concourse.bass as bass
import concourse.mybir as mybir
import concourse.tile as tile
from concourse import bacc
from concourse.bass_utils import run_bass_kernel_spmd

F32 = mybir.dt.float32
F32R = mybir.dt.float32r
BF16 = mybir.dt.bfloat16
FP16 = mybir.dt.float16
ALU = mybir.AluOpType
ACTF = mybir.ActivationFunctionType

P = 128          # partitions / f-chunk size
F = 4096         # dictionary size
T = 64           # signal dim
NCH = F // P     # 32 chunks
B = 128          # batch rows per core
NCORES = 8
MAX_ITER = 25
STEP = np.float32(1.0 / F)
THR = np.float32(0.5) * STEP
GRP = 4          # chunks per elementwise group
NGRP = NCH // GRP
ALAG = 2         # groups of delay before A-chain streams fresh z
KAPPA = 1024.0   # fp16 square pre-scale


def _activation_raw(nc, out, in_, func, bias, scale=1.0):
    """nc.scalar.activation minus the Rsqrt accuracy guard.

    Safe here: rsqrt feeds only the soft-threshold scale, where its error is
    attenuated by thr/mag; the final output magnitude uses Sqrt instead.
    """
    inputs = [nc.scalar.lower_ap(in_)]
    for arg in (bias, scale, 0.0):
        if isinstance(arg, float):
            inputs.append(mybir.ImmediateValue(dtype=F32, value=arg))
        else:
            inputs.append(nc.scalar.lower_ap(arg))
    return nc.scalar.add_instruction(
        mybir.InstActivation(
            name=nc.get_next_instruction_name(),
            func=func,
            ins=inputs,
            outs=[nc.scalar.lower_ap(out)],
        )
    )


def _momentum_scalars():
    """a_j = fp16(1+gamma_j) (exact in fp16), b_j = 1 - a_j (exact fp32)."""
    ts = [1.0]
    for _ in range(MAX_ITER + 2):
        ts.append((1.0 + np.sqrt(1.0 + 4.0 * ts[-1] ** 2)) / 2.0)
    al, bl = [], []
    for j in range(MAX_ITER):
        gam = 0.0 if j < 2 else (ts[j - 1] - 1.0) / ts[j]
        a_hat = float(np.float16(1.0 + gam))
        al.append(a_hat)
        bl.append(float(1.0 - a_hat))
    return al, bl


def build_nc():
    nc = bacc.Bacc(None)
    IDN_d = nc.declare_dram_parameter("IDN", [P, P], F32R, isOutput=False)
    R20_d = nc.declare_dram_parameter("R20", [P, 2 * B], FP16, isOutput=False)
    W2_d = nc.declare_dram_parameter("W2", [P, NCH, P], FP16, isOutput=False)
    X4_d = nc.declare_dram_parameter("X4", [P, 2 * B], F32, isOutput=False)
    SSW_d = nc.declare_dram_parameter("SSW", [P, T], F32R, isOutput=False)
    I64P_d = nc.declare_dram_parameter("I64P", [T, T], FP16, isOutput=False)
    I64N_d = nc.declare_dram_parameter("I64N", [T, T], FP16, isOutput=False)
    W1_d = nc.declare_dram_parameter("W1", [P, NCH, P], F32R, isOutput=False)
    mag_d = nc.declare_dram_parameter("magT", [P, NCH, B], FP16, isOutput=True)

    alphas, betas = _momentum_scalars()

    with tile.TileContext(nc) as tc, ExitStack() as ctx:
        state = ctx.enter_context(tc.tile_pool(name="state", bufs=1))
        temps = ctx.enter_context(tc.tile_pool(name="temps", bufs=3))
        deep = ctx.enter_context(tc.tile_pool(name="deep", bufs=4))
        small = ctx.enter_context(tc.tile_pool(name="small", bufs=2))
        psum_u = ctx.enter_context(tc.tile_pool(name="psum_u", bufs=3, space="PSUM"))
        psum_p1 = ctx.enter_context(tc.tile_pool(name="psum_p1", bufs=1, space="PSUM"))
        psum_b = ctx.enter_context(tc.tile_pool(name="psum_b", bufs=1, space="PSUM"))

        # ---- persistent SBUF tensors
        IDN = state.tile([P, P], F32R, tag="IDN")
        R20 = state.tile([P, 2 * B], FP16, tag="R20")
        W2 = state.tile([P, NCH, P], FP16, tag="W2")
        X4 = state.tile([P, 2 * B], F32, tag="X4")
        SSW = state.tile([P, T], F32R, tag="SSW")
        I64P = state.tile([T, T], FP16, tag="I64P")
        I64N = state.tile([T, T], FP16, tag="I64N")
        W1 = state.tile([P, NCH, P], F32R, tag="W1")
        zA = state.tile([P, NCH, 2 * B], F32R, tag="zA")
        zB = state.tile([P, NCH, 2 * B], F32R, tag="zB")
        aI = state.tile([P, MAX_ITER, P], F32R, tag="aI")
        magT = state.tile([P, NCH, B], FP16, tag="magT")
        one_col = state.tile([P, 1], F32, tag="oc")
        eps_col = state.tile([P, 1], F32, tag="ec")
        zero_col = state.tile([P, 1], F32, tag="zc")

        nc.sync.dma_start(IDN[:], IDN_d[:])
        nc.sync.dma_start(R20[:], R20_d[:])
        nc.sync.dma_start(W2[:], W2_d[:])
        nc.sync.dma_start(X4[:], X4_d[:])
        nc.sync.dma_start(SSW[:], SSW_d[:])
        nc.sync.dma_start(I64P[:], I64P_d[:])
        nc.sync.dma_start(I64N[:], I64N_d[:])
        nc.sync.dma_start(W1[:], W1_d[:])

        nc.vector.memset(one_col[:], 1.0 / KAPPA)
        nc.vector.memset(eps_col[:], 1e-30)
        nc.vector.memset(zero_col[:], 0.0)
        # scaled identities for all iterations, built once upfront
        for j in range(1, MAX_ITER):
            nc.vector.tensor_scalar_mul(aI[:, j, :], IDN[:], KAPPA * alphas[j])

        zbuf = [zA, zB]
        r2cat = None      # fp16 residual stream for the current iteration
        qold = None       # b*P1_old - X4 for the upcoming boundary

        def emit_mom_a(j, g, u_ps):
            aIj = aI[:, j, :]
            for pi in range(GRP // 2):
                c2 = GRP * g + 2 * pi
                out_sl = u_ps[:, 2 * pi:2 * pi + 2, :].rearrange(
                    "p c n -> p (c n)")
                nc.tensor.matmul(
                    out_sl, aIj,
                    zbuf[j % 2][:, c2:c2 + 2, :].rearrange("p c n -> p (c n)"),
                    start=True, stop=False, skip_group_check=True,
                )

        def emit_achain(j, ga, p1a, p1b):
            z_new = zbuf[(j + 1) % 2]
            tgt = p1a if ga < NGRP - 1 else p1b
            c_lo = 0 if ga < NGRP - 1 else NCH - GRP
            c_hi = NCH - GRP - 1 if ga < NGRP - 1 else NCH - 1
            for ci in range(GRP):
                c = GRP * ga + ci
                nc.tensor.matmul(
                    tgt, W1[:, c, :], z_new[:, c, :],
                    start=(c == c_lo), stop=(c == c_hi),
                    skip_group_check=True,
                )

        u_pending = {}  # pre-emitted momentum psum tiles for next iteration

        for j in range(MAX_ITER):
            b = betas[j]
            last = j == MAX_ITER - 1
            z_new = zbuf[(j + 1) % 2]   # holds z_{j-1} until overwritten

            p1a = p1b = None
            if not last:
                p1pair = psum_p1.tile([P, 2, 2 * B], F32, tag="P1")
                p1a = p1pair[:, 0, :]
                p1b = p1pair[:, 1, :]

            pair_m2 = pair_s = None
            pend = []     # (g, u, m2_slice) for the even group of a pair
            pend_rs = []  # pairs awaiting rsq/s (one-pair software pipeline)
            pend_z = []   # pairs awaiting z-writes (another half-pair lag)

            def soft_tail(g, u, m2v, sv):
                """z-write (or mag on last iter) for group g."""
                if not last:
                    z_sl = z_new[:, GRP * g:GRP * (g + 1), :]
                    nc.vector.tensor_tensor(
                        z_sl[:, :, 0:B], u[:, :, 0:B], sv[:], ALU.mult)
                    nc.vector.tensor_tensor(
                        z_sl[:, :, B:2 * B], u[:, :, B:2 * B], sv[:], ALU.mult)
                else:
                    mag = temps.tile([P, GRP, B], F32, tag="mag")
                    nc.scalar.activation(
                        mag[:], m2v[:], ACTF.Sqrt, bias=eps_col[:])
                    nc.vector.tensor_tensor(
                        magT[:, GRP * g:GRP * (g + 1), :], mag[:], sv[:],
                        ALU.mult)
                    nc.sync.dma_start(
                        mag_d[:, GRP * g:GRP * (g + 1), :],
                        magT[:, GRP * g:GRP * (g + 1), :])

            def soft_tail_pair(ge, upair, sv):
                """pair-wide z-write: one TT per re/im half."""
                z_sl = z_new[:, GRP * ge:GRP * (ge + 2), :]
                nc.vector.tensor_tensor(
                    z_sl[:, :, 0:B], upair[:, :, 0:B], sv[:, :, :], ALU.mult)
                nc.gpsimd.tensor_tensor(
                    z_sl[:, :, B:2 * B], upair[:, :, B:2 * B], sv[:, :, :],
                    ALU.mult)

            def emit_rs(ge, ue, m2e, go, uo, m2o, pm2, ps):
                rsq = temps.tile([P, 2 * GRP, B], FP16, tag="rsqp")
                _activation_raw(nc, rsq[:], pm2[:], ACTF.Rsqrt,
                                bias=eps_col[:])
                nc.scalar.activation(
                    ps[:], rsq[:], ACTF.Relu, bias=one_col[:],
                    scale=-float(THR))
                pend_z.append((ge, ue, m2e, go, uo, m2o, ps))

            def emit_z():
                ge, ue, m2e, go, uo, m2o, ps = pend_z.pop(0)
                if not last:
                    upair = ue if ue.shape[1] == 2 * GRP else None
                    if upair is not None:
                        soft_tail_pair(ge, upair, ps)
                    else:
                        soft_tail(ge, ue, m2e, ps[:, 0:GRP, :])
                        soft_tail(go, uo, m2o, ps[:, GRP:2 * GRP, :])
                else:
                    soft_tail(ge, ue, m2e, ps[:, 0:GRP, :])
                    soft_tail(go, uo, m2o, ps[:, GRP:2 * GRP, :])

            defer = True   # all iterations use the pipelined tail path

            for g in range(NGRP):
                if g in u_pending:
                    u_ps = u_pending.pop(g)
                else:
                    u_ps = psum_u.tile([P, GRP, 2 * B], F32, tag="u")
                    if j > 0:
                        emit_mom_a(j, g, u_ps)
                # gradient matmuls (need r2cat)
                rstream = R20 if j == 0 else r2cat
                for ci in range(GRP):
                    c = GRP * g + ci
                    nc.tensor.matmul(
                        u_ps[:, ci, :], W2[:, c, :], rstream[:],
                        start=(j == 0), stop=(j == 0 or ci == GRP - 1),
                        skip_group_check=True,
                    )

                # ---- elementwise chain for group g; for j<2 the b-term
                # is zero, so the drain is a plain copy (keeps u in SBUF and
                # the whole pipeline uniform across iterations)
                if g % 2 == 0:
                    u_pair = deep.tile([P, 2 * GRP, 2 * B], F32,
                                       tag="u_sb")
                u = u_pair[:, (g % 2) * GRP:(g % 2 + 1) * GRP, :]
                if j >= 2:
                    nc.vector.scalar_tensor_tensor(
                        u, z_new[:, GRP * g:GRP * (g + 1), :], KAPPA * b,
                        u_ps[:], ALU.mult, ALU.add,
                    )
                else:
                    nc.vector.tensor_copy(u, u_ps[:])
                # deferred z-writes of an earlier pair go behind this group's
                # drain so the DVE never head-of-line blocks on ACT's s
                if defer and g % 2 == 0 and pend_z:
                    emit_z()
                if g % 2 == 0:
                    pair_m2 = deep.tile([P, 2 * GRP, B], FP16, tag="m2p")
                    pair_s = deep.tile([P, 2 * GRP, B], F32, tag="sp")
                    pend.append((g, u_pair, pair_m2[:, 0:GRP, :]))
                else:
                    t12p = temps.tile([P, 2 * GRP, 2 * B], FP16, tag="t12p")
                    nc.scalar.activation(
                        t12p[:], u_pair[:], ACTF.Square, bias=zero_col[:])
                    nc.gpsimd.tensor_tensor(
                        pair_m2[:], t12p[:, :, 0:B], t12p[:, :, B:2 * B],
                        ALU.add)
                    m2v = pair_m2[:, GRP:2 * GRP, :]
                    ge, ue, m2e = pend.pop()
                    if defer:
                        pend_rs.append((ge, ue, m2e, g, u, m2v,
                                        pair_m2, pair_s))
                        if len(pend_rs) > 1:
                            emit_rs(*pend_rs.pop(0))
                    else:
                        emit_rs(ge, ue, m2e, g, u, m2v, pair_m2, pair_s)
                        emit_z()

            # flush the software pipeline
            while pend_rs:
                emit_rs(*pend_rs.pop(0))
            while pend_z:
                emit_z()

            if last:
                break

            # A-chain for z_{j+1} runs here, after all its z groups are
            # written: the PE never stalls waiting on the elementwise chain.
            for ga in range(NGRP - 1):
                emit_achain(j, ga, p1a, p1b)

            a_n = alphas[j + 1]
            if j == 0:
                qold = small.tile([P, 2 * B], F32, tag="qold")
                nc.gpsimd.tensor_scalar_mul(qold[:], X4[:], -1.0)
            # partial residual combine over chunks 0..27 (off critical path)
            rqp = small.tile([P, 2 * B], F32, tag="rqp")
            nc.vector.scalar_tensor_tensor(
                rqp[:], p1a, a_n, qold[:], ALU.mult, ALU.add)
            if j + 2 < MAX_ITER:
                qoldp = small.tile([P, 2 * B], F32, tag="qoldp")
                nc.vector.scalar_tensor_tensor(
                    qoldp[:], p1a, betas[j + 2], X4[:],
                    ALU.mult, ALU.subtract)

            emit_achain(j, NGRP - 1, p1a, p1b)

            # ---- iteration boundary: build the fp16 residual stream for
            # j+1. P1 quadrants: rows 0:T = [Dr w_r | Dr w_i], rows T:P =
            # [Di w_r | Di w_i]; X4 top rows carry xr|xi. True residual:
            #   r_re = Rq[t, b] - Rq[T+t, B+b],  r_im = Rq[t, B+b] + Rq[T+t, b]
            # Partition realignment runs on the PE: SSW swaps the bottom
            # quadrant rows up; I64P/I64N build the [r_im | -r_re] bottom half.
            rq = small.tile([P, 2 * B], F32R, tag="rq")
            nc.vector.scalar_tensor_tensor(
                rq[:], p1b, a_n, rqp[:], ALU.mult, ALU.add)
            # pre-emit momentum for the first groups of the next iteration:
            # independent PE work that covers the residual-fold latency
            u_pending = {}
            up0 = psum_u.tile([P, GRP, 2 * B], F32, tag="u")
            emit_mom_a(j + 1, 0, up0)
            u_pending[0] = up0
            qb = psum_b.tile([P, 2, 2 * B], F32, tag="qb")
            qsw = qb[:, 0, :]
            nc.tensor.matmul(qsw[0:T, :], SSW[:], rq[:],
                             start=True, stop=True, skip_group_check=True)
            up1 = psum_u.tile([P, GRP, 2 * B], F32, tag="u")
            emit_mom_a(j + 1, 1, up1)
            u_pending[1] = up1
            r2cat = small.tile([P, 2 * B], FP16, tag="r2cat")
            nc.vector.tensor_tensor(
                r2cat[0:T, 0:B], rq[0:T, 0:B], qsw[0:T, B:2 * B], ALU.subtract)
            nc.vector.tensor_tensor(
                r2cat[0:T, B:2 * B], rq[0:T, B:2 * B], qsw[0:T, 0:B], ALU.add)
            bot = qb[:, 1, :]
            nc.tensor.matmul(bot[T:P, 0:B], I64P[:], r2cat[0:T, B:2 * B],
                             start=True, stop=False, skip_group_check=True)
            nc.tensor.matmul(bot[T:P, B:2 * B], I64N[:], r2cat[0:T, 0:B],
                             start=False, stop=True, skip_group_check=True)
            up2 = psum_u.tile([P, GRP, 2 * B], F32, tag="u")
            emit_mom_a(j + 1, 2, up2)
            u_pending[2] = up2
            nc.scalar.copy(r2cat[T:P, :], bot[T:P, :])
            if j + 2 < MAX_ITER:
                qold = small.tile([P, 2 * B], F32, tag="qold")
                nc.vector.scalar_tensor_tensor(
                    qold[:], p1b, betas[j + 2], qoldp[:],
                    ALU.mult, ALU.add)

    nc.finalize()
    return nc


def prep_host_inputs(x, D):
    """Builds per-core input maps from the full inputs."""
    Dr = np.ascontiguousarray(D.real).astype(np.float32)
    Di = np.ascontiguousarray(D.imag).astype(np.float32)
    W1c = np.concatenate(
        [Dr.T.reshape(NCH, P, T), Di.T.reshape(NCH, P, T)], axis=2
    )
    W1 = np.ascontiguousarray(W1c.transpose(1, 0, 2)).astype(np.float32)
    W2 = np.ascontiguousarray(
        KAPPA * np.concatenate([-STEP * Dr, -STEP * Di], axis=0).reshape(P, NCH, P)
    ).astype(np.float16)
    IDN = np.eye(P, dtype=np.float32)
    SSW = np.zeros((P, T), dtype=np.float32)
    for m in range(T):
        SSW[T + m, m] = 1.0
    I64P = np.eye(T, dtype=np.float16)
    I64N = -np.eye(T, dtype=np.float16)

    in_maps = []
    for i in range(NCORES):
        xs = x[i * B:(i + 1) * B]
        xr = np.ascontiguousarray(xs[:, 0].astype(np.float32).T)  # [T, B]
        xi = np.ascontiguousarray(xs[:, 1].astype(np.float32).T)
        X4 = np.zeros((P, 2 * B), dtype=np.float32)
        X4[0:T, 0:B] = xr
        X4[0:T, B:] = xi
        R20 = np.zeros((P, 2 * B), dtype=np.float16)
        R20[0:T, 0:B] = -xr
        R20[0:T, B:] = -xi
        R20[T:P, 0:B] = -xi
        R20[T:P, B:] = xr
        in_maps.append({
            "IDN": IDN, "R20": R20, "W2": W2, "X4": X4, "W1": W1,
            "SSW": SSW, "I64P": I64P, "I64N": I64N,
        })
    return in_maps


def gather_output(results):
    outs = []
    for i in range(NCORES):
        magT = results[i]["magT"].reshape(P, NCH, B).astype(np.float32)
        outs.append(np.ascontiguousarray(magT.transpose(2, 1, 0)).reshape(B, F))
    mag_all = np.concatenate(outs, axis=0)
    return (mag_all / mag_all.max()).astype(np.float32)


_NC_CACHE = {}


def get_nc():
    if "nc" not in _NC_CACHE:
        _NC_CACHE["nc"] = build_nc()
    return _NC_CACHE["nc"]


def kernel(x, D):
    x = np.asarray(x)
    D = np.asarray(D)
    nc = get_nc()
    in_maps = prep_host_inputs(x, D)
    res = run_bass_kernel_spmd(nc, in_maps, list(range(NCORES)))
    return gather_output(res.results)


if __name__ == "__main__":
    import reference as ref
    inputs = ref.setup_inputs()
    out = kernel(**{k: np.asarray(v) for k, v in inputs.items()})
    print("kernel output", out.shape, out.dtype)
